# revision 17
# baseline (speedup 1.0000x reference)
"""Trainium2 Bass kernel: causal multi-head self-attention (streaming v2).

Problem: B=2, T=4096, C=768, H=12, D=64, causal softmax(QK^T/sqrt(D))V + out proj.

Sharding (8 cores): core c handles batch b=c//4 and 3 heads g=c%4 (rows
192*g:192*(g+1) of wq/wk/wv, same columns of wo). Host sums the 4 partial
out-projections per batch.

v2 dataflow (single fused stream, QB=256 query blocks):
  - projections are streamed per 256-column chunk and interleaved with
    attention: proj(ch) emits right before attention q-block qb=ch-1 ends,
    so exp work starts ~5us into the kernel instead of ~100us.
  - scores for heads 0/1 are computed as K=64 row-tiled matmul pairs
    (tile_position (0,0)/(64,0)) running concurrently in the PE array;
    head 2 is self-paired via duplicated K/Q partition halves.
  - PSUM: 6 banks = 2 x [128,1536] score-group slots (2 l-tiles x 3 heads),
    1 bank = ctx accumulators h0|h1 (col halves, rows 0:65 = ctx+denominator),
    1 bank = ctx h2 (cols 0:256) | out-proj staging (cols 256:512).
  - one exp (ScalarE, scale=1/8 folded) per group: N=1536 over 3 banks.
  - causal masking is post-exp: DVE multiply by 0/1 bf16 masks on the
    diagonal group only (keeps the PE free of mask matmuls).
  - ctx matmuls use [V|ones] 65-col stationary so PSUM row 64 accumulates
    the softmax denominator; normalize runs off the critical path.
  - out-proj units are deferred one q-block and popped one per group.
"""

import os
import sys
import types
from collections import deque

import numpy as np

if "/opt/trn_rl_repo" not in sys.path:
    sys.path.insert(0, "/opt/trn_rl_repo")

import concourse.bass as bass  # noqa: E402
import concourse.mybir as mybir  # noqa: E402
from concourse import bacc, tile  # noqa: E402
from concourse.bass_utils import run_bass_kernel_spmd  # noqa: E402

F32 = mybir.dt.float32
F32R = mybir.dt.float32r
BF16 = mybir.dt.bfloat16
EXP = mybir.ActivationFunctionType.Exp

ROWTILE = bool(int(os.environ.get("KERNEL_ROWTILE", "1")))

B, T, C, H, D = 2, 4096, 768, 12, 64
HPD = 3
DH = HPD * D      # 192 local head channels
NCORES = 8
QB = 256          # query block
LT = 128          # key(l)-tile size
NCH = T // QB     # 16 proj chunks / q-blocks
NCT = C // 128    # 6 contraction tiles


def build_kernel(t=T, trace_sim=False):
    n_qb = t // QB
    nch = t // QB
    nct = NCT

    nc = bacc.Bacc("TRN2", target_bir_lowering=False, debug=False,
                   num_devices=NCORES)
    xT_d = nc.dram_tensor("xT", [C, t], F32R, kind="ExternalInput")
    wS_d = nc.dram_tensor("wS", [C, 384], F32R, kind="ExternalInput")
    wvT_d = nc.dram_tensor("wvT", [C, DH], F32R, kind="ExternalInput")
    woT_d = nc.dram_tensor("woT", [256, C], F32R, kind="ExternalInput")
    outT_d = nc.dram_tensor("outT", [C, t], F32, kind="ExternalOutput")

    with tile.TileContext(nc, trace_sim=trace_sim) as tc:
        with (
            tc.tile_pool(name="const", bufs=1) as const,
            tc.tile_pool(name="xs", bufs=3) as xs,
            tc.tile_pool(name="epool", bufs=3) as epool,
            tc.tile_pool(name="small", bufs=3) as small,
            tc.tile_pool(name="otp", bufs=2) as otp,
            tc.tile_pool(name="scp", bufs=2, space="PSUM") as scp,
            tc.tile_pool(name="cpp", bufs=1, space="PSUM") as cpp,
        ):
            # ---- weights ------------------------------------------------
            wS_s = const.tile([128, nct, 384], F32R)
            wvT_s = const.tile([128, nct, DH], F32R)
            nc.sync.dma_start(wS_s[:], wS_d.ap().rearrange("(ct p) d -> p ct d", p=128))
            nc.sync.dma_start(wvT_s[:], wvT_d.ap().rearrange("(ct p) d -> p ct d", p=128))
            woT_a = const.tile([128, C], F32R)
            woT_b = const.tile([128, C], F32R)   # rows 64:128 are host zeros
            nc.sync.dma_start(woT_a[:], woT_d.ap()[0:128, :])
            nc.sync.dma_start(woT_b[:], woT_d.ap()[128:256, :])

            # 0/1 post-exp causal masks (bf16): m0[p,f]=1 iff f>=p ; m1 iff f>=128+p
            m0f = const.tile([128, QB], F32)
            m1f = const.tile([128, QB], F32)
            for mf, base in ((m0f, 0), (m1f, -128)):
                nc.gpsimd.memset(mf[:], 1.0)
                nc.gpsimd.affine_select(
                    out=mf[:], in_=mf[:],
                    compare_op=mybir.AluOpType.is_ge,
                    fill=0.0, base=base, channel_multiplier=-1,
                    pattern=[[1, QB]],
                )
            m0 = const.tile([128, QB], BF16)
            m1 = const.tile([128, QB], BF16)
            nc.vector.tensor_copy(m0[:], m0f[:])
            nc.vector.tensor_copy(m1[:], m1f[:])

            ones1 = const.tile([128, 1], F32)
            nc.vector.memset(ones1[:], 1.0)

            # ---- persistent activations --------------------------------
            QT01 = const.tile([128, t], BF16)   # rows 0:64 h0, 64:128 h1
            KT01 = const.tile([128, t], BF16)
            QT2 = const.tile([128, t], BF16)    # h2 duplicated in both halves
            KT2 = const.tile([128, t], BF16)
            if not ROWTILE:
                # fallback: per-head K tiles zero-padded to K=128 so scores
                # run as plain full-contraction matmuls (dead rows x Q = 0)
                KTz0 = const.tile([128, t], BF16)
                KTz1 = const.tile([128, t], BF16)
            n_lt = t // LT
            Vone = const.tile([128, n_lt, HPD * 65], BF16)
            ctxT01 = const.tile([128, t], F32R)
            ctxT2z = const.tile([128, t], F32R)  # rows 64:128 zeros

            zero1 = const.tile([128, 1], F32)
            nc.vector.memset(zero1[:], 0.0)
            nc.vector.tensor_copy(ctxT2z[:], zero1[:].to_broadcast((128, t)))
            if not ROWTILE:
                nc.vector.tensor_copy(KTz0[64:128, :],
                                      zero1[0:64, :].to_broadcast((64, t)))
                nc.vector.tensor_copy(KTz1[0:64, :],
                                      zero1[0:64, :].to_broadcast((64, t)))
                nc.vector.tensor_copy(KT2[64:128, :],
                                      zero1[0:64, :].to_broadcast((64, t)))
            # ones columns of Vone (index 64 of each head's 65-col block)
            nc.vector.tensor_copy(
                Vone[:].rearrange("p a b -> p (a b)"),
                ones1[:].to_broadcast((128, n_lt * HPD * 65)))

            # ---- projection chunk (contiguous emission) -----------------
            def emit_dma_x(ch):
                if ch >= nch:
                    return
                cs = slice(ch * QB, (ch + 1) * QB)
                xc = xs.tile([128, nct, QB], F32R, tag="xc", name=f"xc{ch}")
                xT_r = xT_d.ap().rearrange("(ct p) t -> p ct t", p=128)
                for ct in range(nct):
                    nc.sync.dma_start(xc[:, ct, :], xT_r[:, ct, cs])
                xcs[ch] = xc

            xcs = {}

            def emit_proj(ch):
                # PSUM group discipline: one start/stop per 2KB bank.
                # bank0 = {q01 | k01}, bank1 = {qk2}, bank2 = {v0 | v1}.
                cs = slice(ch * QB, (ch + 1) * QB)
                emit_dma_x(ch + 2)
                xc = xcs.pop(ch)
                pj = scp.tile([128, 1536], F32, tag="sg", name=f"pj{ch}")
                q01 = pj[:, 0:QB]
                k01 = pj[:, QB:2 * QB]
                qk2 = pj[:, 2 * QB:3 * QB]
                for ct in range(nct):
                    f, l = (ct == 0), (ct == nct - 1)
                    nc.tensor.matmul(q01, wS_s[:, ct, 0:128], xc[:, ct, :],
                                     start=f, stop=False)
                    nc.tensor.matmul(k01, wS_s[:, ct, 128:256], xc[:, ct, :],
                                     start=False, stop=l)
                    nc.tensor.matmul(qk2, wS_s[:, ct, 256:384], xc[:, ct, :],
                                     start=f, stop=l)
                # V natural layout: stationary x block, stream wvT (F=192)
                for ts in range(QB // 128):
                    pv = pj[:, 4 * QB + ts * DH: 4 * QB + (ts + 1) * DH]
                    for ct in range(nct):
                        nc.tensor.matmul(pv, xc[:, ct, ts * 128:(ts + 1) * 128],
                                         wvT_s[:, ct, :],
                                         start=(ct == 0 and ts == 0),
                                         stop=(ct == nct - 1 and ts == 1))
                # copies out of PSUM
                nc.vector.tensor_copy(QT01[:, cs], q01)
                nc.vector.tensor_copy(QT2[0:64, cs], qk2[0:64, :])
                nc.vector.tensor_copy(QT2[64:128, cs], qk2[0:64, :])
                nc.vector.tensor_copy(KT2[0:64, cs], qk2[64:128, :])
                if ROWTILE:
                    nc.vector.tensor_copy(KT01[:, cs], k01)
                    nc.vector.tensor_copy(KT2[64:128, cs], qk2[64:128, :])
                else:
                    nc.vector.tensor_copy(KTz0[0:64, cs], k01[0:64, :])
                    nc.vector.tensor_copy(KTz1[64:128, cs], k01[64:128, :])
                for ts in range(QB // 128):
                    tt = ch * (QB // 128) + ts
                    pv = pj[:, 4 * QB + ts * DH: 4 * QB + (ts + 1) * DH]
                    for h in range(HPD):
                        nc.vector.tensor_copy(
                            Vone[:, tt, h * 65:h * 65 + 64],
                            pv[:, h * 64:(h + 1) * 64])

            # ---- out-projection units ----------------------------------
            def emit_outproj(qb, oc, po):
                qs = slice(qb * QB, (qb + 1) * QB)
                ocs = slice(oc * 128, (oc + 1) * 128)
                nc.tensor.matmul(po, woT_a[:, ocs], ctxT01[:, qs],
                                 start=True, stop=False)
                nc.tensor.matmul(po, woT_b[:, ocs], ctxT2z[:, qs],
                                 start=False, stop=True)
                ot = otp.tile([128, QB], F32, tag="ot")
                nc.vector.tensor_copy(ot[:], po)
                nc.sync.dma_start(outT_d.ap()[ocs, qs], ot[:])

            def emit_outproj_block(qb, banks):
                # 6 units for q-block qb, ping-ponging between the two retired
                # ctx banks (each unit's start=True zeroes its whole bank).
                for oc in range(nct):
                    emit_outproj(qb, oc, banks[oc % 2][:, 0:QB])

            # ---- main stream --------------------------------------------
            emit_dma_x(0)
            emit_dma_x(1)
            emit_proj(0)

            prev_banks = None  # (cp01, cp2) of the previous q-block

            for qb in range(n_qb):
                qs = slice(qb * QB, (qb + 1) * QB)
                if qb + 1 < nch:
                    emit_proj(qb + 1)
                L = 2 * (qb + 1)          # l-tiles for this q-block
                G = L // 2                # groups of 2 l-tiles

                cp01 = cpp.tile([128, 512], F32, tag="c01", name=f"c01_{qb}")
                cp2 = cpp.tile([128, 512], F32, tag="c2p", name=f"c2p_{qb}")

                prev = None  # (et, lt0)

                def emit_ctx(et, lt0):
                    # cp01 holds h0|h1 in column halves under ONE psum group
                    # per q-block (start only on the very first matmul, stop
                    # on the very last); cp2 likewise for h2.
                    lt1 = lt0 + 1
                    for i, lt in enumerate((lt0, lt1)):
                        co = i * QB
                        nc.tensor.matmul(cp01[0:65, 0:QB],
                                         Vone[:, lt, 0:65],
                                         et[:, co:co + QB],
                                         start=(lt == 0), stop=False)
                        nc.tensor.matmul(cp01[0:65, QB:2 * QB],
                                         Vone[:, lt, 65:130],
                                         et[:, 2 * QB + co:2 * QB + co + QB],
                                         start=False, stop=(lt == L - 1))
                        nc.tensor.matmul(cp2[0:65, 0:QB],
                                         Vone[:, lt, 130:195],
                                         et[:, 4 * QB + co:4 * QB + co + QB],
                                         start=(lt == 0), stop=(lt == L - 1))

                for g in range(G):
                    lt0 = 2 * g
                    lt1 = lt0 + 1
                    sgt = scp.tile([128, 1536], F32, tag="sg",
                                   name=f"sg{qb}_{g}")
                    # scores: h0/h1 row-tiled K=64 pairs; h2 self-paired.
                    # per bank: lt0 matmul starts the group, lt1 stops it.
                    for i, lt in enumerate((lt0, lt1)):
                        ls = slice(lt * LT, (lt + 1) * LT)
                        co = i * QB
                        st, sp = (i == 0), (i == 1)
                        if ROWTILE:
                            nc.tensor.matmul(sgt[:, co:co + QB],
                                             KT01[0:64, ls], QT01[0:64, qs],
                                             start=st, stop=sp,
                                             tile_position=(0, 0))
                            nc.tensor.matmul(
                                sgt[:, 2 * QB + co:2 * QB + co + QB],
                                KT01[64:128, ls], QT01[64:128, qs],
                                start=st, stop=sp,
                                tile_position=(64, 0))
                        else:
                            nc.tensor.matmul(sgt[:, co:co + QB],
                                             KTz0[:, ls], QT01[:, qs],
                                             start=st, stop=sp)
                            nc.tensor.matmul(
                                sgt[:, 2 * QB + co:2 * QB + co + QB],
                                KTz1[:, ls], QT01[:, qs],
                                start=st, stop=sp)
                    if ROWTILE:
                        nc.tensor.matmul(sgt[:, 4 * QB:5 * QB],
                                         KT2[0:64, lt0 * LT:(lt0 + 1) * LT],
                                         QT2[0:64, qs],
                                         start=True, stop=False,
                                         tile_position=(0, 0))
                        nc.tensor.matmul(sgt[:, 5 * QB:6 * QB],
                                         KT2[64:128, lt1 * LT:(lt1 + 1) * LT],
                                         QT2[64:128, qs],
                                         start=False, stop=True,
                                         tile_position=(64, 0))
                    else:
                        nc.tensor.matmul(sgt[:, 4 * QB:5 * QB],
                                         KT2[:, lt0 * LT:(lt0 + 1) * LT],
                                         QT2[:, qs],
                                         start=True, stop=False)
                        nc.tensor.matmul(sgt[:, 5 * QB:6 * QB],
                                         KT2[:, lt1 * LT:(lt1 + 1) * LT],
                                         QT2[:, qs],
                                         start=False, stop=True)

                    et = epool.tile([128, 1536], BF16, tag="et")
                    nc.scalar.activation(et[:], sgt[:], EXP, scale=0.125)
                    if g == G - 1:
                        # diagonal group: zero the non-causal region post-exp
                        for r, mm in ((0, m0), (1, m1), (2, m0), (3, m1),
                                      (4, m0), (5, m1)):
                            nc.vector.tensor_mul(et[:, r * QB:(r + 1) * QB],
                                                 et[:, r * QB:(r + 1) * QB],
                                                 mm[:])
                    if g == 0 and prev_banks is not None:
                        # out-proj for qb-1 on the retired ctx banks, before
                        # this q-block's first ctx matmuls claim them
                        emit_outproj_block(qb - 1, prev_banks)
                    if prev is not None:
                        emit_ctx(*prev)
                    prev = (et, lt0)

                emit_ctx(*prev)

                # ---- normalize (off critical path) ----------------------
                dn = small.tile([1, 3 * QB], F32, tag="dn")
                nc.vector.tensor_copy(dn[:, 0:QB], cp01[64:65, 0:QB])
                nc.vector.tensor_copy(dn[:, QB:2 * QB], cp01[64:65, QB:2 * QB])
                nc.vector.tensor_copy(dn[:, 2 * QB:3 * QB], cp2[64:65, 0:QB])
                rec = small.tile([1, 3 * QB], F32, tag="rec")
                nc.vector.reciprocal_approx_fast(rec[:], dn[:])
                rb = small.tile([64, 3 * QB], F32, tag="rb")
                nc.gpsimd.partition_broadcast(rb[:], rec[:])
                nc.vector.tensor_mul(ctxT01[0:64, qs], cp01[0:64, 0:QB],
                                     rb[:, 0:QB])
                st2 = small.tile([64, QB], F32R, tag="st2")
                nc.vector.tensor_mul(st2[:], cp01[0:64, QB:2 * QB],
                                     rb[:, QB:2 * QB])
                nc.sync.dma_start(ctxT01[64:128, qs], st2[:])
                nc.vector.tensor_mul(ctxT2z[0:64, qs], cp2[0:64, 0:QB],
                                     rb[:, 2 * QB:3 * QB])

                prev_banks = (cp2, cp01)

            # drain the final q-block's out-proj units
            emit_outproj_block(n_qb - 1, prev_banks)

    nc.compile()
    return nc


_NC_CACHE = {}
LAST_EXEC_NS = None
LAST_RES = None


def _get_nc():
    if "full" not in _NC_CACHE:
        _NC_CACHE["full"] = build_kernel()
    return _NC_CACHE["full"]


def _install_ntff_shim():
    """Make run_bass_kernel_spmd(trace=True) work under axon in this image."""
    import antenv
    if "antenv.axon_hooks" in sys.modules:
        return
    mod = types.ModuleType("antenv.axon_hooks")
    mod._hook = None
    mod.set_axon_ntff_profile_hook = lambda h: setattr(mod, "_hook", h)
    mod.get_axon_ntff_profile_hook = lambda: mod._hook
    sys.modules["antenv.axon_hooks"] = mod
    antenv.axon_hooks = mod
    try:
        from trn_agent_boot.trn_boot import _ntff_profile_via_ctypes
        mod.set_axon_ntff_profile_hook(
            _ntff_profile_via_ctypes("/opt/axon/libaxon_pjrt.so"))
    except Exception:
        pass


def make_in_maps(x, wq, wk, wv, wo):
    x = np.asarray(x, dtype=np.float32)
    wq = np.asarray(wq, dtype=np.float32)
    wk = np.asarray(wk, dtype=np.float32)
    wv = np.asarray(wv, dtype=np.float32)
    wo = np.asarray(wo, dtype=np.float32)
    in_maps = []
    for c in range(NCORES):
        b, g = c // (NCORES // B), c % (NCORES // B)
        rs, re = g * DH, (g + 1) * DH
        wS = np.empty((C, 384), dtype=np.float32)
        wS[:, 0:128] = wq[rs:rs + 128].T
        wS[:, 128:256] = wk[rs:rs + 128].T
        wS[:, 256:320] = wq[rs + 128:re].T
        wS[:, 320:384] = wk[rs + 128:re].T
        woT = np.zeros((256, C), dtype=np.float32)
        woT[:DH] = wo[:, rs:re].T
        in_maps.append({
            "xT": np.ascontiguousarray(x[b].T),
            "wS": wS,
            "wvT": np.ascontiguousarray(wv[rs:re].T),
            "woT": woT,
        })
    return in_maps


def kernel(x, wq, wk, wv, wo):
    global LAST_EXEC_NS, LAST_RES
    in_maps = make_in_maps(x, wq, wk, wv, wo)
    nc = _get_nc()
    trace = bool(int(os.environ.get("KERNEL_TRACE", "0")))
    if trace:
        try:
            _install_ntff_shim()
        except Exception:
            trace = False
    try:
        res = run_bass_kernel_spmd(nc, in_maps, core_ids=list(range(NCORES)),
                                   trace=trace)
    except Exception:
        if not trace:
            raise
        res = run_bass_kernel_spmd(nc, in_maps, core_ids=list(range(NCORES)),
                                   trace=False)
    LAST_EXEC_NS = res.exec_time_ns
    LAST_RES = res
    outT = [res.results[c]["outT"] for c in range(NCORES)]
    halves = []
    for b in range(B):
        acc = outT[4 * b].astype(np.float64)
        for c in range(4 * b + 1, 4 * b + 4):
            acc = acc + outT[c]
        halves.append(acc.T)
    return np.stack(halves).astype(np.float32)


# revision 18
# speedup vs baseline: 1.2339x; 1.2339x over previous
"""Trainium2 Bass kernel: causal multi-head self-attention (streaming v3).

Problem: B=2, T=4096, C=768, H=12, D=64, causal softmax(QK^T/sqrt(D))V + out proj.

Sharding (8 cores): core c handles batch b=c//4 and 3 heads g=c%4 (rows
192*g:192*(g+1) of wq/wk/wv, same columns of wo). Host sums the 4 partial
out-projections per batch.

v3 dataflow (single fused stream, QB=512 q-blocks, N=512 matmuls):
  - projections stream per 512-column chunk, emitted at q-block boundaries
    so exp work starts ~8us into the kernel instead of ~100us.
  - per q-block, two sub-phases sharing one PSUM slot shape [128,1536]:
      phase A: per l-tile, h0/h1 scores into banks 0/1 (K=128 with
      zero-padded per-head K tiles), one exp N=1024, ctx into 2 rotating
      ctx banks (cx pool).
      phase B: h2 scores for l-tile pairs into banks 0/1, exp N=1024,
      ctx into the cx slot freed by h0's normalize.
  - bank 2 of each score slot carries one deferred out-projection unit
    (its own psum group; start=True zeroes only that bank).
  - causal masking is post-exp: DVE multiply by 0/1 bf16 masks on diagonal
    l-tiles only (no mask matmuls on the PE).
  - ctx matmuls use [V|ones] 65-col stationary so PSUM row 64 accumulates
    the softmax denominator; normalize runs off the critical path; h1's
    normalized ctx reaches partitions 64:128 via a SBUF->SBUF DMA.
"""

import os
import sys
import types
from collections import deque

import numpy as np

if "/opt/trn_rl_repo" not in sys.path:
    sys.path.insert(0, "/opt/trn_rl_repo")

import concourse.bass as bass  # noqa: E402
import concourse.mybir as mybir  # noqa: E402
from concourse import bacc, tile  # noqa: E402
from concourse.bass_utils import run_bass_kernel_spmd  # noqa: E402

F32 = mybir.dt.float32
F32R = mybir.dt.float32r
BF16 = mybir.dt.bfloat16
EXP = mybir.ActivationFunctionType.Exp

B, T, C, H, D = 2, 4096, 768, 12, 64
HPD = 3
DH = HPD * D      # 192 local head channels
NCORES = 8
QB = 512          # query block / proj chunk
LT = 128          # key(l)-tile size
NCT = C // 128    # 6 contraction tiles


def build_kernel(t=T, trace_sim=False):
    n_qb = t // QB
    nch = t // QB
    nct = NCT

    nc = bacc.Bacc("TRN2", target_bir_lowering=False, debug=False,
                   num_devices=NCORES)
    xT_d = nc.dram_tensor("xT", [C, t], F32R, kind="ExternalInput")
    wS_d = nc.dram_tensor("wS", [C, 384], F32R, kind="ExternalInput")
    wvT_d = nc.dram_tensor("wvT", [C, DH], F32R, kind="ExternalInput")
    woT_d = nc.dram_tensor("woT", [256, C], F32R, kind="ExternalInput")
    outT_d = nc.dram_tensor("outT", [C, t], F32, kind="ExternalOutput")

    with tile.TileContext(nc, trace_sim=trace_sim) as tc:
        with (
            tc.tile_pool(name="const", bufs=1) as const,
            tc.tile_pool(name="xs", bufs=3) as xs,
            tc.tile_pool(name="epool", bufs=4) as epool,
            tc.tile_pool(name="small", bufs=3) as small,
            tc.tile_pool(name="otp", bufs=2) as otp,
            tc.tile_pool(name="scp", bufs=2, space="PSUM") as scp,
            tc.tile_pool(name="cxp", bufs=2, space="PSUM") as cxp,
        ):
            # ---- weights ------------------------------------------------
            wS_s = const.tile([128, nct, 384], F32R)
            wvT_s = const.tile([128, nct, DH], F32R)
            nc.sync.dma_start(wS_s[:], wS_d.ap().rearrange("(ct p) d -> p ct d", p=128))
            nc.sync.dma_start(wvT_s[:], wvT_d.ap().rearrange("(ct p) d -> p ct d", p=128))
            woT_a = const.tile([128, C], F32R)
            woT_b = const.tile([128, C], F32R)   # rows 64:128 are host zeros
            nc.sync.dma_start(woT_a[:], woT_d.ap()[0:128, :])
            nc.sync.dma_start(woT_b[:], woT_d.ap()[128:256, :])

            # 0/1 post-exp causal masks: mk[p,f] = 1 iff f >= 128*k + p
            masks = []
            for k in range(QB // LT):
                mf = const.tile([128, QB], F32, tag="mf", name=f"mf{k}")
                nc.gpsimd.memset(mf[:], 1.0)
                nc.gpsimd.affine_select(
                    out=mf[:], in_=mf[:],
                    compare_op=mybir.AluOpType.is_ge,
                    fill=0.0, base=-LT * k, channel_multiplier=-1,
                    pattern=[[1, QB]],
                )
                mb = const.tile([128, QB], BF16, tag=f"mb{k}", name=f"mb{k}")
                nc.vector.tensor_copy(mb[:], mf[:])
                masks.append(mb)

            ones1 = const.tile([128, 1], F32)
            nc.vector.memset(ones1[:], 1.0)
            zero1 = const.tile([128, 1], F32)
            nc.vector.memset(zero1[:], 0.0)

            # ---- persistent activations --------------------------------
            QT01 = const.tile([128, t], BF16)   # rows 0:64 h0, 64:128 h1
            QT2 = const.tile([128, t], BF16)    # h2 duplicated in both halves
            # per-head K tiles zero-padded to K=128 (dead rows x Q = 0)
            KTz0 = const.tile([128, t], BF16)
            KTz1 = const.tile([128, t], BF16)
            KT2z = const.tile([128, t], BF16)
            n_lt = t // LT
            Vone = const.tile([128, n_lt, HPD * 65], BF16)
            ctxT01 = const.tile([128, t], F32R)
            ctxT2z = const.tile([128, t], F32R)  # rows 64:128 zeros

            nc.vector.tensor_copy(ctxT2z[:], zero1[:].to_broadcast((128, t)))
            z64 = zero1[0:64, :]
            nc.vector.tensor_copy(KTz0[64:128, :], z64.to_broadcast((64, t)))
            nc.vector.tensor_copy(KTz1[0:64, :], z64.to_broadcast((64, t)))
            nc.vector.tensor_copy(KT2z[64:128, :], z64.to_broadcast((64, t)))
            # ones columns of Vone (index 64 of each head's 65-col block)
            nc.vector.tensor_copy(
                Vone[:].rearrange("p a b -> p (a b)"),
                ones1[:].to_broadcast((128, n_lt * HPD * 65)))

            # ---- projection chunks --------------------------------------
            xcs = {}

            def emit_dma_x(ch):
                if ch >= nch:
                    return
                cs = slice(ch * QB, (ch + 1) * QB)
                xc = xs.tile([128, nct, QB], F32R, tag="xc", name=f"xc{ch}")
                xT_r = xT_d.ap().rearrange("(ct p) t -> p ct t", p=128)
                for ct in range(nct):
                    nc.sync.dma_start(xc[:, ct, :], xT_r[:, ct, cs])
                xcs[ch] = xc

            def emit_proj(ch):
                cs = slice(ch * QB, (ch + 1) * QB)
                emit_dma_x(ch + 2)
                xc = xcs.pop(ch)
                # slot 1: bank-aligned {q01 | k01 | qk2}
                pj = scp.tile([128, 1536], F32, tag="sg", name=f"pj{ch}")
                q01 = pj[:, 0:QB]
                k01 = pj[:, QB:2 * QB]
                qk2 = pj[:, 2 * QB:3 * QB]
                for ct in range(nct):
                    f, l = (ct == 0), (ct == nct - 1)
                    nc.tensor.matmul(q01, wS_s[:, ct, 0:128], xc[:, ct, :],
                                     start=f, stop=l)
                    nc.tensor.matmul(k01, wS_s[:, ct, 128:256], xc[:, ct, :],
                                     start=f, stop=l)
                    nc.tensor.matmul(qk2, wS_s[:, ct, 256:384], xc[:, ct, :],
                                     start=f, stop=l)
                nc.vector.tensor_copy(QT01[:, cs], q01)
                nc.vector.tensor_copy(QT2[0:64, cs], qk2[0:64, :])
                nc.vector.tensor_copy(QT2[64:128, cs], qk2[0:64, :])
                nc.vector.tensor_copy(KTz0[0:64, cs], k01[0:64, :])
                nc.vector.tensor_copy(KTz1[64:128, cs], k01[64:128, :])
                nc.vector.tensor_copy(KT2z[0:64, cs], qk2[64:128, :])
                # slot 2: V in natural layout (stationary x, stream wvT F=192)
                # regions packed 2-per-bank to stay inside bank boundaries
                pv = scp.tile([128, 1536], F32, tag="sg", name=f"pv{ch}")
                for ts in range(QB // 128):
                    off = (ts // 2) * 512 + (ts % 2) * DH
                    po = pv[:, off:off + DH]
                    for ct in range(nct):
                        nc.tensor.matmul(po, xc[:, ct, ts * 128:(ts + 1) * 128],
                                         wvT_s[:, ct, :],
                                         start=(ct == 0 and ts % 2 == 0),
                                         stop=(ct == nct - 1 and ts % 2 == 1))
                for ts in range(QB // 128):
                    off = (ts // 2) * 512 + (ts % 2) * DH
                    tt = ch * (QB // 128) + ts
                    for h in range(HPD):
                        nc.vector.tensor_copy(
                            Vone[:, tt, h * 65:h * 65 + 64],
                            pv[:, off + h * 64:off + (h + 1) * 64])

            # ---- out-projection units (ride bank 2 of a score slot) -----
            def emit_outproj(qb, oc, po):
                qs = slice(qb * QB, (qb + 1) * QB)
                ocs = slice(oc * 128, (oc + 1) * 128)
                nc.tensor.matmul(po, woT_a[:, ocs], ctxT01[:, qs],
                                 start=True, stop=False)
                nc.tensor.matmul(po, woT_b[:, ocs], ctxT2z[:, qs],
                                 start=False, stop=True)
                ot = otp.tile([128, QB], F32, tag="ot")
                nc.vector.tensor_copy(ot[:], po)
                nc.sync.dma_start(outT_d.ap()[ocs, qs], ot[:])

            pending = deque()

            def pop_outproj(sg):
                if pending:
                    sqb, soc = pending.popleft()
                    emit_outproj(sqb, soc, sg[:, 2 * QB:3 * QB])

            def normalize(cx, col, dst_row01, qs, via_dma):
                # cx[0:65, :]: rows 0:64 ctx, row 64 denominator
                dn = small.tile([1, QB], F32, tag="dn")
                nc.vector.tensor_copy(dn[:], cx[64:65, :])
                rec = small.tile([1, QB], F32, tag="rec")
                nc.vector.reciprocal_approx_fast(rec[:], dn[:])
                rb = small.tile([64, QB], F32, tag="rb")
                nc.gpsimd.partition_broadcast(rb[:], rec[:])
                if via_dma:
                    st2 = small.tile([64, QB], F32R, tag="st2")
                    nc.vector.tensor_mul(st2[:], cx[0:64, :], rb[:])
                    nc.sync.dma_start(dst_row01[64:128, qs], st2[:])
                else:
                    nc.vector.tensor_mul(dst_row01[0:64, qs], cx[0:64, :], rb[:])

            # ---- main stream --------------------------------------------
            emit_dma_x(0)
            emit_dma_x(1)
            emit_proj(0)

            for qb in range(n_qb):
                qs = slice(qb * QB, (qb + 1) * QB)
                if qb + 1 < nch:
                    emit_proj(qb + 1)
                L = 4 * (qb + 1)          # l-tiles for this q-block

                cxA = cxp.tile([128, 512], F32, tag="cx", name=f"cxA{qb}")
                cxB = cxp.tile([128, 512], F32, tag="cx", name=f"cxB{qb}")

                # ---- phase A: heads 0 and 1 -----------------------------
                prev = None

                def emit_ctxA(et, lt):
                    nc.tensor.matmul(cxA[0:65, :], Vone[:, lt, 0:65],
                                     et[:, 0:QB],
                                     start=(lt == 0), stop=(lt == L - 1))
                    nc.tensor.matmul(cxB[0:65, :], Vone[:, lt, 65:130],
                                     et[:, QB:2 * QB],
                                     start=(lt == 0), stop=(lt == L - 1))

                for lt in range(L):
                    ls = slice(lt * LT, (lt + 1) * LT)
                    sg = scp.tile([128, 1536], F32, tag="sg",
                                  name=f"sa{qb}_{lt}")
                    nc.tensor.matmul(sg[:, 0:QB], KTz0[:, ls], QT01[:, qs],
                                     start=True, stop=True)
                    nc.tensor.matmul(sg[:, QB:2 * QB], KTz1[:, ls],
                                     QT01[:, qs], start=True, stop=True)
                    pop_outproj(sg)
                    et = epool.tile([128, 2 * QB], BF16, tag="et")
                    nc.scalar.activation(et[:], sg[:, 0:2 * QB], EXP,
                                         scale=0.125)
                    diag = lt - 4 * qb
                    if diag >= 0:
                        nc.vector.tensor_mul(et[:, 0:QB], et[:, 0:QB],
                                             masks[diag][:])
                        nc.vector.tensor_mul(et[:, QB:2 * QB],
                                             et[:, QB:2 * QB],
                                             masks[diag][:])
                    if prev is not None:
                        emit_ctxA(*prev)
                    prev = (et, lt)
                emit_ctxA(*prev)

                normalize(cxA[0:65, :], 0, ctxT01, qs, via_dma=False)
                normalize(cxB[0:65, :], 0, ctxT01, qs, via_dma=True)

                # ---- phase B: head 2 over l-tile pairs ------------------
                cx2 = cxp.tile([128, 512], F32, tag="cx", name=f"cx2{qb}")
                prev = None

                def emit_ctx2(et, lt0):
                    nc.tensor.matmul(cx2[0:65, :], Vone[:, lt0, 130:195],
                                     et[:, 0:QB],
                                     start=(lt0 == 0), stop=False)
                    nc.tensor.matmul(cx2[0:65, :], Vone[:, lt0 + 1, 130:195],
                                     et[:, QB:2 * QB],
                                     start=False, stop=(lt0 + 1 == L - 1))

                for g in range(L // 2):
                    lt0 = 2 * g
                    sg = scp.tile([128, 1536], F32, tag="sg",
                                  name=f"sb{qb}_{g}")
                    nc.tensor.matmul(sg[:, 0:QB],
                                     KT2z[:, lt0 * LT:(lt0 + 1) * LT],
                                     QT2[:, qs], start=True, stop=True)
                    nc.tensor.matmul(sg[:, QB:2 * QB],
                                     KT2z[:, (lt0 + 1) * LT:(lt0 + 2) * LT],
                                     QT2[:, qs], start=True, stop=True)
                    pop_outproj(sg)
                    et = epool.tile([128, 2 * QB], BF16, tag="et")
                    nc.scalar.activation(et[:], sg[:, 0:2 * QB], EXP,
                                         scale=0.125)
                    for i in range(2):
                        diag = lt0 + i - 4 * qb
                        if diag >= 0:
                            nc.vector.tensor_mul(et[:, i * QB:(i + 1) * QB],
                                                 et[:, i * QB:(i + 1) * QB],
                                                 masks[diag][:])
                    if prev is not None:
                        emit_ctx2(*prev)
                    prev = (et, lt0)
                emit_ctx2(*prev)

                normalize(cx2[0:65, :], 0, ctxT2z, qs, via_dma=False)

                pending.extend((qb, oc) for oc in range(nct))

            # drain remaining out-proj units on fresh slots
            while pending:
                sg = scp.tile([128, 1536], F32, tag="sg", name="sgf")
                pop_outproj(sg)
                if pending:
                    sqb, soc = pending.popleft()
                    emit_outproj(sqb, soc, sg[:, 0:QB])
                if pending:
                    sqb, soc = pending.popleft()
                    emit_outproj(sqb, soc, sg[:, QB:2 * QB])

    nc.compile()
    return nc


_NC_CACHE = {}
LAST_EXEC_NS = None
LAST_RES = None


def _get_nc():
    if "full" not in _NC_CACHE:
        _NC_CACHE["full"] = build_kernel()
    return _NC_CACHE["full"]


def _install_ntff_shim():
    """Make run_bass_kernel_spmd(trace=True) work under axon in this image."""
    import antenv
    if "antenv.axon_hooks" in sys.modules:
        return
    mod = types.ModuleType("antenv.axon_hooks")
    mod._hook = None
    mod.set_axon_ntff_profile_hook = lambda h: setattr(mod, "_hook", h)
    mod.get_axon_ntff_profile_hook = lambda: mod._hook
    sys.modules["antenv.axon_hooks"] = mod
    antenv.axon_hooks = mod
    try:
        from trn_agent_boot.trn_boot import _ntff_profile_via_ctypes
        mod.set_axon_ntff_profile_hook(
            _ntff_profile_via_ctypes("/opt/axon/libaxon_pjrt.so"))
    except Exception:
        pass


def make_in_maps(x, wq, wk, wv, wo):
    x = np.asarray(x, dtype=np.float32)
    wq = np.asarray(wq, dtype=np.float32)
    wk = np.asarray(wk, dtype=np.float32)
    wv = np.asarray(wv, dtype=np.float32)
    wo = np.asarray(wo, dtype=np.float32)
    in_maps = []
    for c in range(NCORES):
        b, g = c // (NCORES // B), c % (NCORES // B)
        rs, re = g * DH, (g + 1) * DH
        wS = np.empty((C, 384), dtype=np.float32)
        wS[:, 0:128] = wq[rs:rs + 128].T
        wS[:, 128:256] = wk[rs:rs + 128].T
        wS[:, 256:320] = wq[rs + 128:re].T
        wS[:, 320:384] = wk[rs + 128:re].T
        woT = np.zeros((256, C), dtype=np.float32)
        woT[:DH] = wo[:, rs:re].T
        in_maps.append({
            "xT": np.ascontiguousarray(x[b].T),
            "wS": wS,
            "wvT": np.ascontiguousarray(wv[rs:re].T),
            "woT": woT,
        })
    return in_maps


def kernel(x, wq, wk, wv, wo):
    global LAST_EXEC_NS, LAST_RES
    in_maps = make_in_maps(x, wq, wk, wv, wo)
    nc = _get_nc()
    trace = bool(int(os.environ.get("KERNEL_TRACE", "0")))
    if trace:
        try:
            _install_ntff_shim()
        except Exception:
            trace = False
    try:
        res = run_bass_kernel_spmd(nc, in_maps, core_ids=list(range(NCORES)),
                                   trace=trace)
    except Exception:
        if not trace:
            raise
        res = run_bass_kernel_spmd(nc, in_maps, core_ids=list(range(NCORES)),
                                   trace=False)
    LAST_EXEC_NS = res.exec_time_ns
    LAST_RES = res
    outT = [res.results[c]["outT"] for c in range(NCORES)]
    halves = []
    for b in range(B):
        acc = outT[4 * b].astype(np.float64)
        for c in range(4 * b + 1, 4 * b + 4):
            acc = acc + outT[c]
        halves.append(acc.T)
    return np.stack(halves).astype(np.float32)


# revision 21
# speedup vs baseline: 1.2615x; 1.0224x over previous
"""Trainium2 Bass kernel: causal multi-head self-attention (streaming v3).

Problem: B=2, T=4096, C=768, H=12, D=64, causal softmax(QK^T/sqrt(D))V + out proj.

Sharding (8 cores): core c handles batch b=c//4 and 3 heads g=c%4 (rows
192*g:192*(g+1) of wq/wk/wv, same columns of wo). Host sums the 4 partial
out-projections per batch.

v3 dataflow (single fused stream, QB=512 q-blocks, N=512 matmuls):
  - projections stream per 512-column chunk, emitted at q-block boundaries
    so exp work starts ~8us into the kernel instead of ~100us.
  - per q-block, two sub-phases sharing one PSUM slot shape [128,1536]:
      phase A: per l-tile, h0/h1 scores into banks 0/1 (K=128 with
      zero-padded per-head K tiles), one exp N=1024, ctx into 2 rotating
      ctx banks (cx pool).
      phase B: h2 scores for l-tile pairs into banks 0/1, exp N=1024,
      ctx into the cx slot freed by h0's normalize.
  - bank 2 of each score slot carries one deferred out-projection unit
    (its own psum group; start=True zeroes only that bank).
  - causal masking is post-exp: DVE multiply by 0/1 bf16 masks on diagonal
    l-tiles only (no mask matmuls on the PE).
  - ctx matmuls use [V|ones] 65-col stationary so PSUM row 64 accumulates
    the softmax denominator; normalize runs off the critical path; h1's
    normalized ctx reaches partitions 64:128 via a SBUF->SBUF DMA.
"""

import os
import sys
import types
from collections import deque

import ml_dtypes
import numpy as np

if "/opt/trn_rl_repo" not in sys.path:
    sys.path.insert(0, "/opt/trn_rl_repo")

import concourse.bass as bass  # noqa: E402
import concourse.mybir as mybir  # noqa: E402
from concourse import bacc, tile  # noqa: E402
from concourse.bass_utils import run_bass_kernel_spmd  # noqa: E402

F32 = mybir.dt.float32
F32R = mybir.dt.float32r
BF16 = mybir.dt.bfloat16
EXP = mybir.ActivationFunctionType.Exp

B, T, C, H, D = 2, 4096, 768, 12, 64
HPD = 3
DH = HPD * D      # 192 local head channels
NCORES = 8
QB = 512          # query block / proj chunk
LT = 128          # key(l)-tile size
NCT = C // 128    # 6 contraction tiles


def build_kernel(t=T, trace_sim=False):
    n_qb = t // QB
    nch = t // QB
    nct = NCT

    nc = bacc.Bacc("TRN2", target_bir_lowering=False, debug=False,
                   num_devices=NCORES)
    xT_d = nc.dram_tensor("xT", [C, t], BF16, kind="ExternalInput")
    wS_d = nc.dram_tensor("wS", [C, 384], BF16, kind="ExternalInput")
    wvT_d = nc.dram_tensor("wvT", [C, DH], BF16, kind="ExternalInput")
    woT_d = nc.dram_tensor("woT", [256, C], BF16, kind="ExternalInput")
    outT_d = nc.dram_tensor("outT", [C, t], F32, kind="ExternalOutput")

    with tile.TileContext(nc, trace_sim=trace_sim) as tc:
        with (
            tc.tile_pool(name="const", bufs=1) as const,
            tc.tile_pool(name="xs", bufs=3) as xs,
            tc.tile_pool(name="epool", bufs=4) as epool,
            tc.tile_pool(name="small", bufs=3) as small,
            tc.tile_pool(name="otp", bufs=2) as otp,
            tc.tile_pool(name="scp", bufs=2, space="PSUM") as scp,
            tc.tile_pool(name="cxp", bufs=2, space="PSUM") as cxp,
        ):
            # ---- weights ------------------------------------------------
            wS_s = const.tile([128, nct, 384], BF16)
            wvT_s = const.tile([128, nct, DH], BF16)
            nc.scalar.dma_start(wS_s[:], wS_d.ap().rearrange("(ct p) d -> p ct d", p=128))
            nc.scalar.dma_start(wvT_s[:], wvT_d.ap().rearrange("(ct p) d -> p ct d", p=128))
            woT_a = const.tile([128, C], BF16)
            woT_b = const.tile([128, C], BF16)   # rows 64:128 are host zeros
            nc.scalar.dma_start(woT_a[:], woT_d.ap()[0:128, :])
            nc.scalar.dma_start(woT_b[:], woT_d.ap()[128:256, :])

            # 0/1 post-exp causal masks: mk[p,f] = 1 iff f >= 128*k + p
            masks = []
            for k in range(QB // LT):
                mf = const.tile([128, QB], F32, tag="mf", name=f"mf{k}")
                nc.gpsimd.memset(mf[:], 1.0)
                nc.gpsimd.affine_select(
                    out=mf[:], in_=mf[:],
                    compare_op=mybir.AluOpType.is_ge,
                    fill=0.0, base=-LT * k, channel_multiplier=-1,
                    pattern=[[1, QB]],
                )
                mb = const.tile([128, QB], BF16, tag=f"mb{k}", name=f"mb{k}")
                nc.vector.tensor_copy(mb[:], mf[:])
                masks.append(mb)

            ones1 = const.tile([128, 1], F32)
            nc.vector.memset(ones1[:], 1.0)
            zero1 = const.tile([128, 1], F32)
            nc.vector.memset(zero1[:], 0.0)

            # ---- persistent activations --------------------------------
            QT01 = const.tile([128, t], BF16)   # rows 0:64 h0, 64:128 h1
            QT2 = const.tile([128, t], BF16)    # h2 duplicated in both halves
            # per-head K tiles zero-padded to K=128 (dead rows x Q = 0)
            KTz0 = const.tile([128, t], BF16)
            KTz1 = const.tile([128, t], BF16)
            KT2z = const.tile([128, t], BF16)
            n_lt = t // LT
            Vone = const.tile([128, n_lt, HPD * 65], BF16)
            ctxT01 = const.tile([128, t], BF16)
            ctxT2z = const.tile([128, t], BF16)  # rows 64:128 zeros

            nc.vector.tensor_copy(ctxT2z[:], zero1[:].to_broadcast((128, t)))
            z64 = zero1[0:64, :]
            nc.vector.tensor_copy(KTz0[64:128, :], z64.to_broadcast((64, t)))
            nc.vector.tensor_copy(KTz1[0:64, :], z64.to_broadcast((64, t)))
            nc.vector.tensor_copy(KT2z[64:128, :], z64.to_broadcast((64, t)))
            # ones columns of Vone (index 64 of each head's 65-col block)
            nc.vector.tensor_copy(
                Vone[:].rearrange("p a b -> p (a b)"),
                ones1[:].to_broadcast((128, n_lt * HPD * 65)))

            # ---- projection chunks --------------------------------------
            xcs = {}

            def emit_dma_x(ch):
                if ch >= nch:
                    return
                cs = slice(ch * QB, (ch + 1) * QB)
                xc = xs.tile([128, nct, QB], BF16, tag="xc", name=f"xc{ch}")
                xT_r = xT_d.ap().rearrange("(ct p) t -> p ct t", p=128)
                for ct in range(nct):
                    nc.sync.dma_start(xc[:, ct, :], xT_r[:, ct, cs])
                xcs[ch] = xc

            def emit_proj(ch):
                cs = slice(ch * QB, (ch + 1) * QB)
                emit_dma_x(ch + 2)
                xc = xcs.pop(ch)
                # slot 1: bank-aligned {q01 | k01 | qk2}
                pj = scp.tile([128, 1536], F32, tag="sg", name=f"pj{ch}")
                q01 = pj[:, 0:QB]
                k01 = pj[:, QB:2 * QB]
                qk2 = pj[:, 2 * QB:3 * QB]
                for ct in range(nct):
                    f, l = (ct == 0), (ct == nct - 1)
                    nc.tensor.matmul(q01, wS_s[:, ct, 0:128], xc[:, ct, :],
                                     start=f, stop=l)
                    nc.tensor.matmul(k01, wS_s[:, ct, 128:256], xc[:, ct, :],
                                     start=f, stop=l)
                    nc.tensor.matmul(qk2, wS_s[:, ct, 256:384], xc[:, ct, :],
                                     start=f, stop=l)
                nc.vector.tensor_copy(QT01[:, cs], q01)
                nc.vector.tensor_copy(QT2[0:64, cs], qk2[0:64, :])
                nc.vector.tensor_copy(QT2[64:128, cs], qk2[0:64, :])
                nc.vector.tensor_copy(KTz0[0:64, cs], k01[0:64, :])
                nc.vector.tensor_copy(KTz1[64:128, cs], k01[64:128, :])
                nc.vector.tensor_copy(KT2z[0:64, cs], qk2[64:128, :])
                # slot 2: V in natural layout (stationary x, stream wvT F=192)
                # regions packed 2-per-bank to stay inside bank boundaries
                pv = scp.tile([128, 1536], F32, tag="sg", name=f"pv{ch}")
                for ts in range(QB // 128):
                    off = (ts // 2) * 512 + (ts % 2) * DH
                    po = pv[:, off:off + DH]
                    for ct in range(nct):
                        nc.tensor.matmul(po, xc[:, ct, ts * 128:(ts + 1) * 128],
                                         wvT_s[:, ct, :],
                                         start=(ct == 0 and ts % 2 == 0),
                                         stop=(ct == nct - 1 and ts % 2 == 1))
                for ts in range(QB // 128):
                    off = (ts // 2) * 512 + (ts % 2) * DH
                    tt = ch * (QB // 128) + ts
                    for h in range(HPD):
                        nc.vector.tensor_copy(
                            Vone[:, tt, h * 65:h * 65 + 64],
                            pv[:, off + h * 64:off + (h + 1) * 64])

            # ---- out-projection units (ride bank 2 of a score slot) -----
            def emit_outproj(qb, oc, po):
                qs = slice(qb * QB, (qb + 1) * QB)
                ocs = slice(oc * 128, (oc + 1) * 128)
                nc.tensor.matmul(po, woT_a[:, ocs], ctxT01[:, qs],
                                 start=True, stop=False)
                nc.tensor.matmul(po, woT_b[:, ocs], ctxT2z[:, qs],
                                 start=False, stop=True)
                ot = otp.tile([128, QB], F32, tag="ot")
                nc.vector.tensor_copy(ot[:], po)
                nc.sync.dma_start(outT_d.ap()[ocs, qs], ot[:])

            pending = deque()

            def pop_outproj(sg):
                if pending:
                    sqb, soc = pending.popleft()
                    emit_outproj(sqb, soc, sg[:, 2 * QB:3 * QB])

            def normalize(cx, dst_row01, qs, via_dma):
                # cx[0:65, :]: rows 0:64 ctx, row 64 denominator
                dn = small.tile([1, QB], F32, tag="dn")
                nc.vector.tensor_copy(dn[:], cx[64:65, :])
                rec = small.tile([1, QB], F32, tag="rec")
                nc.vector.reciprocal_approx_fast(rec[:], dn[:])
                rb = small.tile([64, QB], F32, tag="rb")
                nc.gpsimd.partition_broadcast(rb[:], rec[:])
                if via_dma:
                    st2 = small.tile([64, QB], BF16, tag="st2")
                    nc.vector.tensor_mul(st2[:], cx[0:64, :], rb[:])
                    nc.sync.dma_start(dst_row01[64:128, qs], st2[:])
                else:
                    nc.vector.tensor_mul(dst_row01[0:64, qs], cx[0:64, :], rb[:])

            # ---- main stream --------------------------------------------
            # ctx matmuls (and phase-tail normalizes) for slot g are emitted
            # only after slot g+1's exp, so the PE always has the next
            # scores queued while it waits -- including across phase and
            # q-block boundaries.
            ctx_cb = [None]

            def flush_cb():
                cb, ctx_cb[0] = ctx_cb[0], None
                if cb is not None:
                    cb()

            emit_dma_x(0)
            emit_dma_x(1)
            emit_proj(0)

            for qb in range(n_qb):
                qs = slice(qb * QB, (qb + 1) * QB)
                if qb + 1 < nch:
                    emit_proj(qb + 1)
                L = 4 * (qb + 1)          # l-tiles for this q-block

                cxA = cxp.tile([128, 512], F32, tag="cx", name=f"cxA{qb}")
                cxB = cxp.tile([128, 512], F32, tag="cx", name=f"cxB{qb}")

                # ---- phase A: heads 0 and 1 -----------------------------
                def make_ctxA(et, lt, L=L, cxA=cxA, cxB=cxB, qs=qs):
                    def cb():
                        nc.tensor.matmul(cxA[0:65, :], Vone[:, lt, 0:65],
                                         et[:, 0:QB],
                                         start=(lt == 0), stop=(lt == L - 1))
                        nc.tensor.matmul(cxB[0:65, :], Vone[:, lt, 65:130],
                                         et[:, QB:2 * QB],
                                         start=(lt == 0), stop=(lt == L - 1))
                        if lt == L - 1:
                            normalize(cxA[0:65, :], ctxT01, qs, via_dma=False)
                            normalize(cxB[0:65, :], ctxT01, qs, via_dma=True)
                    return cb

                for lt in range(L):
                    ls = slice(lt * LT, (lt + 1) * LT)
                    sg = scp.tile([128, 1536], F32, tag="sg",
                                  name=f"sa{qb}_{lt}")
                    nc.tensor.matmul(sg[:, 0:QB], KTz0[:, ls], QT01[:, qs],
                                     start=True, stop=True)
                    nc.tensor.matmul(sg[:, QB:2 * QB], KTz1[:, ls],
                                     QT01[:, qs], start=True, stop=True)
                    pop_outproj(sg)
                    et = epool.tile([128, 2 * QB], BF16, tag="et")
                    nc.scalar.activation(et[:], sg[:, 0:2 * QB], EXP,
                                         scale=0.125)
                    diag = lt - 4 * qb
                    if diag >= 0:
                        nc.vector.tensor_mul(et[:, 0:QB], et[:, 0:QB],
                                             masks[diag][:])
                        nc.vector.tensor_mul(et[:, QB:2 * QB],
                                             et[:, QB:2 * QB],
                                             masks[diag][:])
                    flush_cb()
                    ctx_cb[0] = make_ctxA(et, lt)

                # ---- phase B: head 2 over l-tile pairs ------------------
                cx2 = cxp.tile([128, 512], F32, tag="cx", name=f"cx2{qb}")

                def make_ctx2(et, lt0, L=L, cx2=cx2, qs=qs, qb=qb):
                    def cb():
                        nc.tensor.matmul(cx2[0:65, :], Vone[:, lt0, 130:195],
                                         et[:, 0:QB],
                                         start=(lt0 == 0), stop=False)
                        nc.tensor.matmul(cx2[0:65, :],
                                         Vone[:, lt0 + 1, 130:195],
                                         et[:, QB:2 * QB],
                                         start=False,
                                         stop=(lt0 + 1 == L - 1))
                        if lt0 + 1 == L - 1:
                            normalize(cx2[0:65, :], ctxT2z, qs, via_dma=False)
                            pending.extend((qb, oc) for oc in range(nct))
                    return cb

                for g in range(L // 2):
                    lt0 = 2 * g
                    sg = scp.tile([128, 1536], F32, tag="sg",
                                  name=f"sb{qb}_{g}")
                    nc.tensor.matmul(sg[:, 0:QB],
                                     KT2z[:, lt0 * LT:(lt0 + 1) * LT],
                                     QT2[:, qs], start=True, stop=True)
                    nc.tensor.matmul(sg[:, QB:2 * QB],
                                     KT2z[:, (lt0 + 1) * LT:(lt0 + 2) * LT],
                                     QT2[:, qs], start=True, stop=True)
                    pop_outproj(sg)
                    et = epool.tile([128, 2 * QB], BF16, tag="et")
                    nc.scalar.activation(et[:], sg[:, 0:2 * QB], EXP,
                                         scale=0.125)
                    for i in range(2):
                        diag = lt0 + i - 4 * qb
                        if diag >= 0:
                            nc.vector.tensor_mul(et[:, i * QB:(i + 1) * QB],
                                                 et[:, i * QB:(i + 1) * QB],
                                                 masks[diag][:])
                    flush_cb()
                    ctx_cb[0] = make_ctx2(et, lt0)

            flush_cb()

            # drain remaining out-proj units on fresh slots
            while pending:
                sg = scp.tile([128, 1536], F32, tag="sg", name="sgf")
                pop_outproj(sg)
                if pending:
                    sqb, soc = pending.popleft()
                    emit_outproj(sqb, soc, sg[:, 0:QB])
                if pending:
                    sqb, soc = pending.popleft()
                    emit_outproj(sqb, soc, sg[:, QB:2 * QB])

    nc.compile()
    return nc


_NC_CACHE = {}
LAST_EXEC_NS = None
LAST_RES = None


def _get_nc():
    if "full" not in _NC_CACHE:
        _NC_CACHE["full"] = build_kernel()
    return _NC_CACHE["full"]


def _install_ntff_shim():
    """Make run_bass_kernel_spmd(trace=True) work under axon in this image."""
    import antenv
    if "antenv.axon_hooks" in sys.modules:
        return
    mod = types.ModuleType("antenv.axon_hooks")
    mod._hook = None
    mod.set_axon_ntff_profile_hook = lambda h: setattr(mod, "_hook", h)
    mod.get_axon_ntff_profile_hook = lambda: mod._hook
    sys.modules["antenv.axon_hooks"] = mod
    antenv.axon_hooks = mod
    try:
        from trn_agent_boot.trn_boot import _ntff_profile_via_ctypes
        mod.set_axon_ntff_profile_hook(
            _ntff_profile_via_ctypes("/opt/axon/libaxon_pjrt.so"))
    except Exception:
        pass


def make_in_maps(x, wq, wk, wv, wo):
    x = np.asarray(x, dtype=np.float32)
    wq = np.asarray(wq, dtype=np.float32)
    wk = np.asarray(wk, dtype=np.float32)
    wv = np.asarray(wv, dtype=np.float32)
    wo = np.asarray(wo, dtype=np.float32)
    in_maps = []
    for c in range(NCORES):
        b, g = c // (NCORES // B), c % (NCORES // B)
        rs, re = g * DH, (g + 1) * DH
        wS = np.empty((C, 384), dtype=np.float32)
        wS[:, 0:128] = wq[rs:rs + 128].T
        wS[:, 128:256] = wk[rs:rs + 128].T
        wS[:, 256:320] = wq[rs + 128:re].T
        wS[:, 320:384] = wk[rs + 128:re].T
        woT = np.zeros((256, C), dtype=np.float32)
        woT[:DH] = wo[:, rs:re].T
        in_maps.append({
            "xT": np.ascontiguousarray(x[b].T).astype(ml_dtypes.bfloat16),
            "wS": wS.astype(ml_dtypes.bfloat16),
            "wvT": np.ascontiguousarray(wv[rs:re].T).astype(ml_dtypes.bfloat16),
            "woT": woT.astype(ml_dtypes.bfloat16),
        })
    return in_maps


def kernel(x, wq, wk, wv, wo):
    global LAST_EXEC_NS, LAST_RES
    in_maps = make_in_maps(x, wq, wk, wv, wo)
    nc = _get_nc()
    trace = bool(int(os.environ.get("KERNEL_TRACE", "0")))
    if trace:
        try:
            _install_ntff_shim()
        except Exception:
            trace = False
    try:
        res = run_bass_kernel_spmd(nc, in_maps, core_ids=list(range(NCORES)),
                                   trace=trace)
    except Exception:
        if not trace:
            raise
        res = run_bass_kernel_spmd(nc, in_maps, core_ids=list(range(NCORES)),
                                   trace=False)
    LAST_EXEC_NS = res.exec_time_ns
    LAST_RES = res
    outT = [res.results[c]["outT"] for c in range(NCORES)]
    halves = []
    for b in range(B):
        acc = outT[4 * b].astype(np.float64)
        for c in range(4 * b + 1, 4 * b + 4):
            acc = acc + outT[c]
        halves.append(acc.T)
    return np.stack(halves).astype(np.float32)


# revision 24
# speedup vs baseline: 1.2626x; 1.0009x over previous
"""Trainium2 Bass kernel: causal multi-head self-attention (streaming v3).

Problem: B=2, T=4096, C=768, H=12, D=64, causal softmax(QK^T/sqrt(D))V + out proj.

Sharding (8 cores): core c handles batch b=c//4 and 3 heads g=c%4 (rows
192*g:192*(g+1) of wq/wk/wv, same columns of wo). Host sums the 4 partial
out-projections per batch.

v3 dataflow (single fused stream, QB=512 q-blocks, N=512 matmuls):
  - projections stream per 512-column chunk, emitted at q-block boundaries
    so exp work starts ~8us into the kernel instead of ~100us.
  - per q-block, two sub-phases sharing one PSUM slot shape [128,1536]:
      phase A: per l-tile, h0/h1 scores into banks 0/1 (K=128 with
      zero-padded per-head K tiles), one exp N=1024, ctx into 2 rotating
      ctx banks (cx pool).
      phase B: h2 scores for l-tile pairs into banks 0/1, exp N=1024,
      ctx into the cx slot freed by h0's normalize.
  - bank 2 of each score slot carries one deferred out-projection unit
    (its own psum group; start=True zeroes only that bank).
  - causal masking is post-exp: DVE multiply by 0/1 bf16 masks on diagonal
    l-tiles only (no mask matmuls on the PE).
  - ctx matmuls use [V|ones] 65-col stationary so PSUM row 64 accumulates
    the softmax denominator; normalize runs off the critical path; h1's
    normalized ctx reaches partitions 64:128 via a SBUF->SBUF DMA.
"""

import os
import sys
import types
from collections import deque

import ml_dtypes
import numpy as np

if "/opt/trn_rl_repo" not in sys.path:
    sys.path.insert(0, "/opt/trn_rl_repo")

import concourse.bass as bass  # noqa: E402
import concourse.mybir as mybir  # noqa: E402
from concourse import bacc, tile  # noqa: E402
from concourse.bass_utils import run_bass_kernel_spmd  # noqa: E402

F32 = mybir.dt.float32
F32R = mybir.dt.float32r
BF16 = mybir.dt.bfloat16
EXP = mybir.ActivationFunctionType.Exp

B, T, C, H, D = 2, 4096, 768, 12, 64
HPD = 3
DH = HPD * D      # 192 local head channels
NCORES = 8
QB = 512          # query block / proj chunk
LT = 128          # key(l)-tile size
NCT = C // 128    # 6 contraction tiles


def build_kernel(t=T, trace_sim=False):
    n_qb = t // QB
    nch = t // QB
    nct = NCT

    nc = bacc.Bacc("TRN2", target_bir_lowering=False, debug=False,
                   num_devices=NCORES)
    xT_d = nc.dram_tensor("xT", [C, t], BF16, kind="ExternalInput")
    wS_d = nc.dram_tensor("wS", [C, 384], BF16, kind="ExternalInput")
    wvT_d = nc.dram_tensor("wvT", [C, DH], BF16, kind="ExternalInput")
    woT_d = nc.dram_tensor("woT", [256, C], BF16, kind="ExternalInput")
    outT_d = nc.dram_tensor("outT", [C, t], F32, kind="ExternalOutput")

    with tile.TileContext(nc, trace_sim=trace_sim) as tc:
        with (
            tc.tile_pool(name="const", bufs=1) as const,
            tc.tile_pool(name="xs", bufs=3) as xs,
            tc.tile_pool(name="epool", bufs=4) as epool,
            tc.tile_pool(name="small", bufs=3) as small,
            tc.tile_pool(name="otp", bufs=2) as otp,
            tc.tile_pool(name="scp", bufs=2, space="PSUM") as scp,
            tc.tile_pool(name="cxp", bufs=2, space="PSUM") as cxp,
        ):
            # ---- weights ------------------------------------------------
            wS_s = const.tile([128, nct, 384], BF16)
            wvT_s = const.tile([128, nct, DH], BF16)
            nc.scalar.dma_start(wS_s[:], wS_d.ap().rearrange("(ct p) d -> p ct d", p=128))
            nc.scalar.dma_start(wvT_s[:], wvT_d.ap().rearrange("(ct p) d -> p ct d", p=128))
            woT_a = const.tile([128, C], BF16)
            woT_b = const.tile([128, C], BF16)   # rows 64:128 are host zeros
            nc.scalar.dma_start(woT_a[:], woT_d.ap()[0:128, :])
            nc.scalar.dma_start(woT_b[:], woT_d.ap()[128:256, :])

            # 0/1 post-exp causal masks: mk[p,f] = 1 iff f >= 128*k + p
            masks = []
            for k in range(QB // LT):
                mf = const.tile([128, QB], F32, tag="mf", name=f"mf{k}")
                nc.gpsimd.memset(mf[:], 1.0)
                nc.gpsimd.affine_select(
                    out=mf[:], in_=mf[:],
                    compare_op=mybir.AluOpType.is_ge,
                    fill=0.0, base=-LT * k, channel_multiplier=-1,
                    pattern=[[1, QB]],
                )
                mb = const.tile([128, QB], BF16, tag=f"mb{k}", name=f"mb{k}")
                nc.vector.tensor_copy(mb[:], mf[:])
                masks.append(mb)

            ones1 = const.tile([128, 1], F32)
            nc.vector.memset(ones1[:], 1.0)
            zero1 = const.tile([128, 1], F32)
            nc.vector.memset(zero1[:], 0.0)

            # ---- persistent activations --------------------------------
            QT01 = const.tile([128, t], BF16)   # rows 0:64 h0, 64:128 h1
            QT2 = const.tile([128, t], BF16)    # h2 duplicated in both halves
            # per-head K tiles zero-padded to K=128 (dead rows x Q = 0)
            KTz0 = const.tile([128, t], BF16)
            KTz1 = const.tile([128, t], BF16)
            KT2z = const.tile([128, t], BF16)
            n_lt = t // LT
            Vone = const.tile([128, n_lt, HPD * 65], BF16)
            ctxT01 = const.tile([128, t], BF16)
            ctxT2z = const.tile([128, t], BF16)  # rows 64:128 zeros


            def emit_fills():
                z64 = zero1[0:64, :]
                nc.vector.tensor_copy(KTz0[64:128, :], z64.to_broadcast((64, t)))
                nc.vector.tensor_copy(KTz1[0:64, :], z64.to_broadcast((64, t)))
                nc.vector.tensor_copy(KT2z[64:128, :], z64.to_broadcast((64, t)))
                for h in range(HPD):
                    nc.vector.tensor_copy(
                        Vone[:, :, h * 65 + 64:h * 65 + 65].rearrange(
                            "p a b -> p (a b)"),
                        ones1[:].to_broadcast((128, n_lt)))
                nc.vector.tensor_copy(ctxT2z[:],
                                      zero1[:].to_broadcast((128, t)))

            # ---- projection chunks --------------------------------------
            xcs = {}

            def emit_dma_x(ch):
                if ch >= nch:
                    return
                cs = slice(ch * QB, (ch + 1) * QB)
                xc = xs.tile([128, nct, QB], BF16, tag="xc", name=f"xc{ch}")
                xT_r = xT_d.ap().rearrange("(ct p) t -> p ct t", p=128)
                for ct in range(nct):
                    nc.sync.dma_start(xc[:, ct, :], xT_r[:, ct, cs])
                xcs[ch] = xc

            def emit_proj(ch):
                cs = slice(ch * QB, (ch + 1) * QB)
                emit_dma_x(ch + 2)
                xc = xcs.pop(ch)
                # slot 1: bank-aligned {q01 | k01 | qk2}
                pj = scp.tile([128, 1536], F32, tag="sg", name=f"pj{ch}")
                q01 = pj[:, 0:QB]
                k01 = pj[:, QB:2 * QB]
                qk2 = pj[:, 2 * QB:3 * QB]
                for ct in range(nct):
                    f, l = (ct == 0), (ct == nct - 1)
                    nc.tensor.matmul(q01, wS_s[:, ct, 0:128], xc[:, ct, :],
                                     start=f, stop=l)
                    nc.tensor.matmul(k01, wS_s[:, ct, 128:256], xc[:, ct, :],
                                     start=f, stop=l)
                    nc.tensor.matmul(qk2, wS_s[:, ct, 256:384], xc[:, ct, :],
                                     start=f, stop=l)
                nc.vector.tensor_copy(QT01[:, cs], q01)
                nc.vector.tensor_copy(QT2[0:64, cs], qk2[0:64, :])
                nc.vector.tensor_copy(QT2[64:128, cs], qk2[0:64, :])
                nc.vector.tensor_copy(KTz0[0:64, cs], k01[0:64, :])
                nc.vector.tensor_copy(KTz1[64:128, cs], k01[64:128, :])
                nc.vector.tensor_copy(KT2z[0:64, cs], qk2[64:128, :])
                # slot 2: V in natural layout (stationary x, stream wvT F=192)
                # regions packed 2-per-bank to stay inside bank boundaries
                pv = scp.tile([128, 1536], F32, tag="sg", name=f"pv{ch}")
                for ts in range(QB // 128):
                    off = (ts // 2) * 512 + (ts % 2) * DH
                    po = pv[:, off:off + DH]
                    for ct in range(nct):
                        nc.tensor.matmul(po, xc[:, ct, ts * 128:(ts + 1) * 128],
                                         wvT_s[:, ct, :],
                                         start=(ct == 0 and ts % 2 == 0),
                                         stop=(ct == nct - 1 and ts % 2 == 1))
                for ts in range(QB // 128):
                    off = (ts // 2) * 512 + (ts % 2) * DH
                    tt = ch * (QB // 128) + ts
                    for h in range(HPD):
                        nc.vector.tensor_copy(
                            Vone[:, tt, h * 65:h * 65 + 64],
                            pv[:, off + h * 64:off + (h + 1) * 64])

            # ---- out-projection units (ride bank 2 of a score slot) -----
            def emit_outproj(qb, oc, po):
                qs = slice(qb * QB, (qb + 1) * QB)
                ocs = slice(oc * 128, (oc + 1) * 128)
                nc.tensor.matmul(po, woT_a[:, ocs], ctxT01[:, qs],
                                 start=True, stop=False)
                nc.tensor.matmul(po, woT_b[:, ocs], ctxT2z[:, qs],
                                 start=False, stop=True)
                ot = otp.tile([128, QB], F32, tag="ot")
                nc.vector.tensor_copy(ot[:], po)
                nc.sync.dma_start(outT_d.ap()[ocs, qs], ot[:])

            pending = deque()

            def pop_outproj(sg):
                if pending:
                    sqb, soc = pending.popleft()
                    emit_outproj(sqb, soc, sg[:, 2 * QB:3 * QB])

            def normalize(cx, dst_row01, qs, via_dma):
                # cx[0:65, :]: rows 0:64 ctx, row 64 denominator
                # (reciprocal must read SBUF: PSUM-src gives garbage on HW)
                dn = small.tile([1, QB], F32, tag="dn")
                nc.vector.tensor_copy(dn[:], cx[64:65, :])
                rec = small.tile([1, QB], F32, tag="rec")
                nc.vector.reciprocal_approx_fast(rec[:], dn[:])
                rb = small.tile([64, QB], F32, tag="rb")
                nc.gpsimd.partition_broadcast(rb[:], rec[:])
                if via_dma:
                    st2 = small.tile([64, QB], BF16, tag="st2")
                    nc.vector.tensor_mul(st2[:], cx[0:64, :], rb[:])
                    nc.sync.dma_start(dst_row01[64:128, qs], st2[:])
                else:
                    nc.vector.tensor_mul(dst_row01[0:64, qs], cx[0:64, :], rb[:])

            # ---- main stream --------------------------------------------
            # ctx matmuls (and phase-tail normalizes) for slot g are emitted
            # only after slot g+1's exp, so the PE always has the next
            # scores queued while it waits -- including across phase and
            # q-block boundaries.
            ctx_cb = [None]

            def flush_cb():
                cb, ctx_cb[0] = ctx_cb[0], None
                if cb is not None:
                    cb()

            emit_dma_x(0)
            emit_dma_x(1)
            emit_proj(0)
            emit_fills()
            # front-load proj chunks: two per q-block while the softmax
            # pipeline is still ramping, none in the PE-bound tail
            proj_sched = {0: (1, 2), 1: (3, 4), 2: (5, 6), 3: (7,)}

            for qb in range(n_qb):
                qs = slice(qb * QB, (qb + 1) * QB)
                for ch in proj_sched.get(qb, ()):
                    if ch < nch:
                        emit_proj(ch)
                L = 4 * (qb + 1)          # l-tiles for this q-block

                cxA = cxp.tile([128, 512], F32, tag="cx", name=f"cxA{qb}")
                cxB = cxp.tile([128, 512], F32, tag="cx", name=f"cxB{qb}")

                # ---- phase A: heads 0 and 1 -----------------------------
                def make_ctxA(et, lt, L=L, cxA=cxA, cxB=cxB, qs=qs):
                    def cb():
                        nc.tensor.matmul(cxA[0:65, :], Vone[:, lt, 0:65],
                                         et[:, 0:QB],
                                         start=(lt == 0), stop=(lt == L - 1))
                        nc.tensor.matmul(cxB[0:65, :], Vone[:, lt, 65:130],
                                         et[:, QB:2 * QB],
                                         start=(lt == 0), stop=(lt == L - 1))
                        if lt == L - 1:
                            normalize(cxA[0:65, :], ctxT01, qs, via_dma=False)
                            normalize(cxB[0:65, :], ctxT01, qs, via_dma=True)
                    return cb

                for lt in range(L):
                    ls = slice(lt * LT, (lt + 1) * LT)
                    sg = scp.tile([128, 1536], F32, tag="sg",
                                  name=f"sa{qb}_{lt}")
                    nc.tensor.matmul(sg[:, 0:QB], KTz0[:, ls], QT01[:, qs],
                                     start=True, stop=True)
                    nc.tensor.matmul(sg[:, QB:2 * QB], KTz1[:, ls],
                                     QT01[:, qs], start=True, stop=True)
                    pop_outproj(sg)
                    et = epool.tile([128, 2 * QB], BF16, tag="et")
                    nc.scalar.activation(et[:], sg[:, 0:2 * QB], EXP,
                                         scale=0.125)
                    diag = lt - 4 * qb
                    if diag >= 0:
                        nc.vector.tensor_mul(et[:, 0:QB], et[:, 0:QB],
                                             masks[diag][:])
                        nc.vector.tensor_mul(et[:, QB:2 * QB],
                                             et[:, QB:2 * QB],
                                             masks[diag][:])
                    flush_cb()
                    ctx_cb[0] = make_ctxA(et, lt)

                # ---- phase B: head 2 over l-tile pairs ------------------
                cx2 = cxp.tile([128, 512], F32, tag="cx", name=f"cx2{qb}")

                def make_ctx2(et, lt0, L=L, cx2=cx2, qs=qs, qb=qb):
                    def cb():
                        nc.tensor.matmul(cx2[0:65, :], Vone[:, lt0, 130:195],
                                         et[:, 0:QB],
                                         start=(lt0 == 0), stop=False)
                        nc.tensor.matmul(cx2[0:65, :],
                                         Vone[:, lt0 + 1, 130:195],
                                         et[:, QB:2 * QB],
                                         start=False,
                                         stop=(lt0 + 1 == L - 1))
                        if lt0 + 1 == L - 1:
                            normalize(cx2[0:65, :], ctxT2z, qs, via_dma=False)
                            pending.extend((qb, oc) for oc in range(nct))
                    return cb

                for g in range(L // 2):
                    lt0 = 2 * g
                    sg = scp.tile([128, 1536], F32, tag="sg",
                                  name=f"sb{qb}_{g}")
                    nc.tensor.matmul(sg[:, 0:QB],
                                     KT2z[:, lt0 * LT:(lt0 + 1) * LT],
                                     QT2[:, qs], start=True, stop=True)
                    nc.tensor.matmul(sg[:, QB:2 * QB],
                                     KT2z[:, (lt0 + 1) * LT:(lt0 + 2) * LT],
                                     QT2[:, qs], start=True, stop=True)
                    pop_outproj(sg)
                    et = epool.tile([128, 2 * QB], BF16, tag="et")
                    nc.scalar.activation(et[:], sg[:, 0:2 * QB], EXP,
                                         scale=0.125)
                    for i in range(2):
                        diag = lt0 + i - 4 * qb
                        if diag >= 0:
                            nc.vector.tensor_mul(et[:, i * QB:(i + 1) * QB],
                                                 et[:, i * QB:(i + 1) * QB],
                                                 masks[diag][:])
                    flush_cb()
                    ctx_cb[0] = make_ctx2(et, lt0)

            flush_cb()

            # drain remaining out-proj units on fresh slots
            while pending:
                sg = scp.tile([128, 1536], F32, tag="sg", name="sgf")
                pop_outproj(sg)
                if pending:
                    sqb, soc = pending.popleft()
                    emit_outproj(sqb, soc, sg[:, 0:QB])
                if pending:
                    sqb, soc = pending.popleft()
                    emit_outproj(sqb, soc, sg[:, QB:2 * QB])

    nc.compile()
    return nc


_NC_CACHE = {}
LAST_EXEC_NS = None
LAST_RES = None


def _get_nc():
    if "full" not in _NC_CACHE:
        _NC_CACHE["full"] = build_kernel()
    return _NC_CACHE["full"]


def _install_ntff_shim():
    """Make run_bass_kernel_spmd(trace=True) work under axon in this image."""
    import antenv
    if "antenv.axon_hooks" in sys.modules:
        return
    mod = types.ModuleType("antenv.axon_hooks")
    mod._hook = None
    mod.set_axon_ntff_profile_hook = lambda h: setattr(mod, "_hook", h)
    mod.get_axon_ntff_profile_hook = lambda: mod._hook
    sys.modules["antenv.axon_hooks"] = mod
    antenv.axon_hooks = mod
    try:
        from trn_agent_boot.trn_boot import _ntff_profile_via_ctypes
        mod.set_axon_ntff_profile_hook(
            _ntff_profile_via_ctypes("/opt/axon/libaxon_pjrt.so"))
    except Exception:
        pass


def make_in_maps(x, wq, wk, wv, wo):
    x = np.asarray(x, dtype=np.float32)
    wq = np.asarray(wq, dtype=np.float32)
    wk = np.asarray(wk, dtype=np.float32)
    wv = np.asarray(wv, dtype=np.float32)
    wo = np.asarray(wo, dtype=np.float32)
    in_maps = []
    for c in range(NCORES):
        b, g = c // (NCORES // B), c % (NCORES // B)
        rs, re = g * DH, (g + 1) * DH
        wS = np.empty((C, 384), dtype=np.float32)
        wS[:, 0:128] = wq[rs:rs + 128].T
        wS[:, 128:256] = wk[rs:rs + 128].T
        wS[:, 256:320] = wq[rs + 128:re].T
        wS[:, 320:384] = wk[rs + 128:re].T
        woT = np.zeros((256, C), dtype=np.float32)
        woT[:DH] = wo[:, rs:re].T
        in_maps.append({
            "xT": np.ascontiguousarray(x[b].T).astype(ml_dtypes.bfloat16),
            "wS": wS.astype(ml_dtypes.bfloat16),
            "wvT": np.ascontiguousarray(wv[rs:re].T).astype(ml_dtypes.bfloat16),
            "woT": woT.astype(ml_dtypes.bfloat16),
        })
    return in_maps


def kernel(x, wq, wk, wv, wo):
    global LAST_EXEC_NS, LAST_RES
    in_maps = make_in_maps(x, wq, wk, wv, wo)
    nc = _get_nc()
    trace = bool(int(os.environ.get("KERNEL_TRACE", "0")))
    if trace:
        try:
            _install_ntff_shim()
        except Exception:
            trace = False
    try:
        res = run_bass_kernel_spmd(nc, in_maps, core_ids=list(range(NCORES)),
                                   trace=trace)
    except Exception:
        if not trace:
            raise
        res = run_bass_kernel_spmd(nc, in_maps, core_ids=list(range(NCORES)),
                                   trace=False)
    LAST_EXEC_NS = res.exec_time_ns
    LAST_RES = res
    outT = [res.results[c]["outT"] for c in range(NCORES)]
    halves = []
    for b in range(B):
        acc = outT[4 * b].astype(np.float64)
        for c in range(4 * b + 1, 4 * b + 4):
            acc = acc + outT[c]
        halves.append(acc.T)
    return np.stack(halves).astype(np.float32)


# revision 25
# speedup vs baseline: 1.3305x; 1.0538x over previous
"""Trainium2 Bass kernel: causal multi-head self-attention (streaming v3).

Problem: B=2, T=4096, C=768, H=12, D=64, causal softmax(QK^T/sqrt(D))V + out proj.

Sharding (8 cores): core c handles batch b=c//4 and 3 heads g=c%4 (rows
192*g:192*(g+1) of wq/wk/wv, same columns of wo). Host sums the 4 partial
out-projections per batch.

v3 dataflow (single fused stream, QB=512 q-blocks, N=512 matmuls):
  - projections stream per 512-column chunk, emitted at q-block boundaries
    so exp work starts ~8us into the kernel instead of ~100us.
  - per q-block, two sub-phases sharing one PSUM slot shape [128,1536]:
      phase A: per l-tile, h0/h1 scores into banks 0/1 (K=128 with
      zero-padded per-head K tiles), one exp N=1024, ctx into 2 rotating
      ctx banks (cx pool).
      phase B: h2 scores for l-tile pairs into banks 0/1, exp N=1024,
      ctx into the cx slot freed by h0's normalize.
  - bank 2 of each score slot carries one deferred out-projection unit
    (its own psum group; start=True zeroes only that bank).
  - causal masking is post-exp: DVE multiply by 0/1 bf16 masks on diagonal
    l-tiles only (no mask matmuls on the PE).
  - ctx matmuls use [V|ones] 65-col stationary so PSUM row 64 accumulates
    the softmax denominator; normalize runs off the critical path; h1's
    normalized ctx reaches partitions 64:128 via a SBUF->SBUF DMA.
"""

import os
import sys
import types
from collections import deque

import ml_dtypes
import numpy as np

if "/opt/trn_rl_repo" not in sys.path:
    sys.path.insert(0, "/opt/trn_rl_repo")

import concourse.bass as bass  # noqa: E402
import concourse.mybir as mybir  # noqa: E402
from concourse import bacc, tile  # noqa: E402
from concourse.bass_utils import run_bass_kernel_spmd  # noqa: E402

F32 = mybir.dt.float32
F32R = mybir.dt.float32r
BF16 = mybir.dt.bfloat16
EXP = mybir.ActivationFunctionType.Exp

B, T, C, H, D = 2, 4096, 768, 12, 64
HPD = 3
DH = HPD * D      # 192 local head channels
NCORES = 8
QB = 512          # query block / proj chunk
LT = 128          # key(l)-tile size
NCT = C // 128    # 6 contraction tiles


def build_kernel(t=T, trace_sim=False):
    n_qb = t // QB
    nch = t // QB
    nct = NCT

    nc = bacc.Bacc("TRN2", target_bir_lowering=False, debug=False,
                   num_devices=NCORES)
    xT_d = nc.dram_tensor("xT", [C, t], BF16, kind="ExternalInput")
    wS_d = nc.dram_tensor("wS", [C, 384], BF16, kind="ExternalInput")
    wvT_d = nc.dram_tensor("wvT", [C, DH], BF16, kind="ExternalInput")
    woT_d = nc.dram_tensor("woT", [256, C], BF16, kind="ExternalInput")
    outT_d = nc.dram_tensor("outT", [C, t], F32, kind="ExternalOutput")

    with tile.TileContext(nc, trace_sim=trace_sim) as tc:
        with (
            tc.tile_pool(name="const", bufs=1) as const,
            tc.tile_pool(name="xs", bufs=3) as xs,
            tc.tile_pool(name="epool", bufs=4) as epool,
            tc.tile_pool(name="small", bufs=3) as small,
            tc.tile_pool(name="otp", bufs=2) as otp,
            tc.tile_pool(name="scp", bufs=2, space="PSUM") as scp,
            tc.tile_pool(name="cxp", bufs=2, space="PSUM") as cxp,
        ):
            # ---- weights ------------------------------------------------
            wS_s = const.tile([128, nct, 384], BF16)
            wvT_s = const.tile([128, nct, DH], BF16)
            nc.scalar.dma_start(wS_s[:], wS_d.ap().rearrange("(ct p) d -> p ct d", p=128))
            nc.scalar.dma_start(wvT_s[:], wvT_d.ap().rearrange("(ct p) d -> p ct d", p=128))
            woT_a = const.tile([128, C], BF16)
            woT_b = const.tile([128, C], BF16)   # rows 64:128 are host zeros
            nc.scalar.dma_start(woT_a[:], woT_d.ap()[0:128, :])
            nc.scalar.dma_start(woT_b[:], woT_d.ap()[128:256, :])

            # 0/1 post-exp causal masks: mk[p,f] = 1 iff f >= 128*k + p
            masks = []
            for k in range(QB // LT):
                mf = const.tile([128, QB], F32, tag="mf", name=f"mf{k}")
                nc.gpsimd.memset(mf[:], 1.0)
                nc.gpsimd.affine_select(
                    out=mf[:], in_=mf[:],
                    compare_op=mybir.AluOpType.is_ge,
                    fill=0.0, base=-LT * k, channel_multiplier=-1,
                    pattern=[[1, QB]],
                )
                mb = const.tile([128, QB], BF16, tag=f"mb{k}", name=f"mb{k}")
                nc.vector.tensor_copy(mb[:], mf[:])
                masks.append(mb)

            ones1 = const.tile([128, 1], F32)
            nc.vector.memset(ones1[:], 1.0)
            zero1 = const.tile([128, 1], F32)
            nc.vector.memset(zero1[:], 0.0)

            # ---- persistent activations --------------------------------
            QT01 = const.tile([128, t], BF16)   # rows 0:64 h0, 64:128 h1
            QT2 = const.tile([128, t], BF16)    # h2 duplicated in both halves
            # per-head K tiles zero-padded to K=128 (dead rows x Q = 0)
            KTz0 = const.tile([128, t], BF16)
            KTz1 = const.tile([128, t], BF16)
            KT2z = const.tile([128, t], BF16)
            n_lt = t // LT
            Vone = const.tile([128, n_lt, HPD * 65], BF16)
            ctxT01 = const.tile([128, t], BF16)
            ctxT2z = const.tile([128, t], BF16)  # rows 64:128 zeros


            def emit_fills():
                z64 = zero1[0:64, :]
                nc.vector.tensor_copy(KTz0[64:128, :], z64.to_broadcast((64, t)))
                nc.vector.tensor_copy(KTz1[0:64, :], z64.to_broadcast((64, t)))
                nc.vector.tensor_copy(KT2z[64:128, :], z64.to_broadcast((64, t)))
                for h in range(HPD):
                    nc.vector.tensor_copy(
                        Vone[:, :, h * 65 + 64:h * 65 + 65].rearrange(
                            "p a b -> p (a b)"),
                        ones1[:].to_broadcast((128, n_lt)))
                nc.vector.tensor_copy(ctxT2z[:],
                                      zero1[:].to_broadcast((128, t)))

            # ---- projection chunks --------------------------------------
            xcs = {}

            def emit_dma_x(ch):
                if ch >= nch:
                    return
                cs = slice(ch * QB, (ch + 1) * QB)
                xc = xs.tile([128, nct, QB], BF16, tag="xc", name=f"xc{ch}")
                xT_r = xT_d.ap().rearrange("(ct p) t -> p ct t", p=128)
                for ct in range(nct):
                    nc.sync.dma_start(xc[:, ct, :], xT_r[:, ct, cs])
                xcs[ch] = xc

            def emit_proj(ch):
                cs = slice(ch * QB, (ch + 1) * QB)
                emit_dma_x(ch + 2)
                xc = xcs.pop(ch)
                # slot 1: bank-aligned {q01 | k01 | qk2}
                pj = scp.tile([128, 1536], F32, tag="sg", name=f"pj{ch}")
                q01 = pj[:, 0:QB]
                k01 = pj[:, QB:2 * QB]
                qk2 = pj[:, 2 * QB:3 * QB]
                for ct in range(nct):
                    f, l = (ct == 0), (ct == nct - 1)
                    nc.tensor.matmul(q01, wS_s[:, ct, 0:128], xc[:, ct, :],
                                     start=f, stop=l)
                    nc.tensor.matmul(k01, wS_s[:, ct, 128:256], xc[:, ct, :],
                                     start=f, stop=l)
                    nc.tensor.matmul(qk2, wS_s[:, ct, 256:384], xc[:, ct, :],
                                     start=f, stop=l)
                nc.scalar.copy(QT01[:, cs], q01)
                nc.scalar.copy(QT2[0:64, cs], qk2[0:64, :])
                nc.scalar.copy(QT2[64:128, cs], qk2[0:64, :])
                nc.scalar.copy(KTz0[0:64, cs], k01[0:64, :])
                nc.scalar.copy(KTz1[64:128, cs], k01[64:128, :])
                nc.scalar.copy(KT2z[0:64, cs], qk2[64:128, :])
                # slot 2: V in natural layout (stationary x, stream wvT F=192)
                # regions packed 2-per-bank to stay inside bank boundaries
                pv = scp.tile([128, 1536], F32, tag="sg", name=f"pv{ch}")
                for ts in range(QB // 128):
                    off = (ts // 2) * 512 + (ts % 2) * DH
                    po = pv[:, off:off + DH]
                    for ct in range(nct):
                        nc.tensor.matmul(po, xc[:, ct, ts * 128:(ts + 1) * 128],
                                         wvT_s[:, ct, :],
                                         start=(ct == 0 and ts % 2 == 0),
                                         stop=(ct == nct - 1 and ts % 2 == 1))
                for ts in range(QB // 128):
                    off = (ts // 2) * 512 + (ts % 2) * DH
                    tt = ch * (QB // 128) + ts
                    for h in range(HPD):
                        nc.vector.tensor_copy(
                            Vone[:, tt, h * 65:h * 65 + 64],
                            pv[:, off + h * 64:off + (h + 1) * 64])

            # ---- out-projection units (ride bank 2 of a score slot) -----
            def emit_outproj(qb, oc, po):
                qs = slice(qb * QB, (qb + 1) * QB)
                ocs = slice(oc * 128, (oc + 1) * 128)
                nc.tensor.matmul(po, woT_a[:, ocs], ctxT01[:, qs],
                                 start=True, stop=False)
                nc.tensor.matmul(po, woT_b[:, ocs], ctxT2z[:, qs],
                                 start=False, stop=True)
                ot = otp.tile([128, QB], F32, tag="ot")
                nc.vector.tensor_copy(ot[:], po)
                nc.sync.dma_start(outT_d.ap()[ocs, qs], ot[:])

            pending = deque()

            def pop_outproj(sg):
                if pending:
                    sqb, soc = pending.popleft()
                    emit_outproj(sqb, soc, sg[:, 2 * QB:3 * QB])

            def normalize(cx, dst_row01, qs, via_dma):
                # cx[0:65, :]: rows 0:64 ctx, row 64 denominator
                # (reciprocal must read SBUF: PSUM-src gives garbage on HW)
                dn = small.tile([1, QB], F32, tag="dn")
                nc.vector.tensor_copy(dn[:], cx[64:65, :])
                rec = small.tile([1, QB], F32, tag="rec")
                nc.vector.reciprocal_approx_fast(rec[:], dn[:])
                rb = small.tile([64, QB], F32, tag="rb")
                nc.gpsimd.partition_broadcast(rb[:], rec[:])
                if via_dma:
                    st2 = small.tile([64, QB], BF16, tag="st2")
                    nc.vector.tensor_mul(st2[:], cx[0:64, :], rb[:])
                    nc.sync.dma_start(dst_row01[64:128, qs], st2[:])
                else:
                    nc.vector.tensor_mul(dst_row01[0:64, qs], cx[0:64, :], rb[:])

            # ---- main stream --------------------------------------------
            # ctx matmuls (and phase-tail normalizes) for slot g are emitted
            # only after slot g+1's exp, so the PE always has the next
            # scores queued while it waits -- including across phase and
            # q-block boundaries.
            ctx_cb = [None]

            def flush_cb():
                cb, ctx_cb[0] = ctx_cb[0], None
                if cb is not None:
                    cb()

            emit_dma_x(0)
            emit_dma_x(1)
            emit_proj(0)
            emit_fills()
            # front-load proj chunks: two per q-block while the softmax
            # pipeline is still ramping, none in the PE-bound tail
            proj_sched = {0: (1,), 1: (2, 3), 2: (4, 5), 3: (6,), 4: (7,)}

            for qb in range(n_qb):
                qs = slice(qb * QB, (qb + 1) * QB)
                for ch in proj_sched.get(qb, ()):
                    if ch < nch:
                        emit_proj(ch)
                L = 4 * (qb + 1)          # l-tiles for this q-block

                cxA = cxp.tile([128, 512], F32, tag="cx", name=f"cxA{qb}")
                cxB = cxp.tile([128, 512], F32, tag="cx", name=f"cxB{qb}")

                # ---- phase A: heads 0 and 1 -----------------------------
                def make_ctxA(et, lt, L=L, cxA=cxA, cxB=cxB, qs=qs):
                    def cb():
                        nc.tensor.matmul(cxA[0:65, :], Vone[:, lt, 0:65],
                                         et[:, 0:QB],
                                         start=(lt == 0), stop=(lt == L - 1))
                        nc.tensor.matmul(cxB[0:65, :], Vone[:, lt, 65:130],
                                         et[:, QB:2 * QB],
                                         start=(lt == 0), stop=(lt == L - 1))
                        if lt == L - 1:
                            normalize(cxA[0:65, :], ctxT01, qs, via_dma=False)
                            normalize(cxB[0:65, :], ctxT01, qs, via_dma=True)
                    return cb

                for lt in range(L):
                    ls = slice(lt * LT, (lt + 1) * LT)
                    sg = scp.tile([128, 1536], F32, tag="sg",
                                  name=f"sa{qb}_{lt}")
                    nc.tensor.matmul(sg[:, 0:QB], KTz0[:, ls], QT01[:, qs],
                                     start=True, stop=True)
                    nc.tensor.matmul(sg[:, QB:2 * QB], KTz1[:, ls],
                                     QT01[:, qs], start=True, stop=True)
                    pop_outproj(sg)
                    et = epool.tile([128, 2 * QB], BF16, tag="et")
                    nc.scalar.activation(et[:], sg[:, 0:2 * QB], EXP,
                                         scale=0.125)
                    diag = lt - 4 * qb
                    if diag >= 0:
                        nc.vector.tensor_mul(et[:, 0:QB], et[:, 0:QB],
                                             masks[diag][:])
                        nc.vector.tensor_mul(et[:, QB:2 * QB],
                                             et[:, QB:2 * QB],
                                             masks[diag][:])
                    flush_cb()
                    ctx_cb[0] = make_ctxA(et, lt)

                # ---- phase B: head 2 over l-tile pairs ------------------
                cx2 = cxp.tile([128, 512], F32, tag="cx", name=f"cx2{qb}")

                def make_ctx2(et, lt0, L=L, cx2=cx2, qs=qs, qb=qb):
                    def cb():
                        nc.tensor.matmul(cx2[0:65, :], Vone[:, lt0, 130:195],
                                         et[:, 0:QB],
                                         start=(lt0 == 0), stop=False)
                        nc.tensor.matmul(cx2[0:65, :],
                                         Vone[:, lt0 + 1, 130:195],
                                         et[:, QB:2 * QB],
                                         start=False,
                                         stop=(lt0 + 1 == L - 1))
                        if lt0 + 1 == L - 1:
                            normalize(cx2[0:65, :], ctxT2z, qs, via_dma=False)
                            pending.extend((qb, oc) for oc in range(nct))
                    return cb

                for g in range(L // 2):
                    lt0 = 2 * g
                    sg = scp.tile([128, 1536], F32, tag="sg",
                                  name=f"sb{qb}_{g}")
                    nc.tensor.matmul(sg[:, 0:QB],
                                     KT2z[:, lt0 * LT:(lt0 + 1) * LT],
                                     QT2[:, qs], start=True, stop=True)
                    nc.tensor.matmul(sg[:, QB:2 * QB],
                                     KT2z[:, (lt0 + 1) * LT:(lt0 + 2) * LT],
                                     QT2[:, qs], start=True, stop=True)
                    pop_outproj(sg)
                    et = epool.tile([128, 2 * QB], BF16, tag="et")
                    nc.scalar.activation(et[:], sg[:, 0:2 * QB], EXP,
                                         scale=0.125)
                    for i in range(2):
                        diag = lt0 + i - 4 * qb
                        if diag >= 0:
                            nc.vector.tensor_mul(et[:, i * QB:(i + 1) * QB],
                                                 et[:, i * QB:(i + 1) * QB],
                                                 masks[diag][:])
                    flush_cb()
                    ctx_cb[0] = make_ctx2(et, lt0)

            flush_cb()

            # drain remaining out-proj units on fresh slots
            while pending:
                sg = scp.tile([128, 1536], F32, tag="sg", name="sgf")
                pop_outproj(sg)
                if pending:
                    sqb, soc = pending.popleft()
                    emit_outproj(sqb, soc, sg[:, 0:QB])
                if pending:
                    sqb, soc = pending.popleft()
                    emit_outproj(sqb, soc, sg[:, QB:2 * QB])

    nc.compile()
    return nc


_NC_CACHE = {}
LAST_EXEC_NS = None
LAST_RES = None


def _get_nc():
    if "full" not in _NC_CACHE:
        _NC_CACHE["full"] = build_kernel()
    return _NC_CACHE["full"]


def _install_ntff_shim():
    """Make run_bass_kernel_spmd(trace=True) work under axon in this image."""
    import antenv
    if "antenv.axon_hooks" in sys.modules:
        return
    mod = types.ModuleType("antenv.axon_hooks")
    mod._hook = None
    mod.set_axon_ntff_profile_hook = lambda h: setattr(mod, "_hook", h)
    mod.get_axon_ntff_profile_hook = lambda: mod._hook
    sys.modules["antenv.axon_hooks"] = mod
    antenv.axon_hooks = mod
    try:
        from trn_agent_boot.trn_boot import _ntff_profile_via_ctypes
        mod.set_axon_ntff_profile_hook(
            _ntff_profile_via_ctypes("/opt/axon/libaxon_pjrt.so"))
    except Exception:
        pass


def make_in_maps(x, wq, wk, wv, wo):
    x = np.asarray(x, dtype=np.float32)
    wq = np.asarray(wq, dtype=np.float32)
    wk = np.asarray(wk, dtype=np.float32)
    wv = np.asarray(wv, dtype=np.float32)
    wo = np.asarray(wo, dtype=np.float32)
    in_maps = []
    for c in range(NCORES):
        b, g = c // (NCORES // B), c % (NCORES // B)
        rs, re = g * DH, (g + 1) * DH
        wS = np.empty((C, 384), dtype=np.float32)
        wS[:, 0:128] = wq[rs:rs + 128].T
        wS[:, 128:256] = wk[rs:rs + 128].T
        wS[:, 256:320] = wq[rs + 128:re].T
        wS[:, 320:384] = wk[rs + 128:re].T
        woT = np.zeros((256, C), dtype=np.float32)
        woT[:DH] = wo[:, rs:re].T
        in_maps.append({
            "xT": np.ascontiguousarray(x[b].T).astype(ml_dtypes.bfloat16),
            "wS": wS.astype(ml_dtypes.bfloat16),
            "wvT": np.ascontiguousarray(wv[rs:re].T).astype(ml_dtypes.bfloat16),
            "woT": woT.astype(ml_dtypes.bfloat16),
        })
    return in_maps


def kernel(x, wq, wk, wv, wo):
    global LAST_EXEC_NS, LAST_RES
    in_maps = make_in_maps(x, wq, wk, wv, wo)
    nc = _get_nc()
    trace = bool(int(os.environ.get("KERNEL_TRACE", "0")))
    if trace:
        try:
            _install_ntff_shim()
        except Exception:
            trace = False
    try:
        res = run_bass_kernel_spmd(nc, in_maps, core_ids=list(range(NCORES)),
                                   trace=trace)
    except Exception:
        if not trace:
            raise
        res = run_bass_kernel_spmd(nc, in_maps, core_ids=list(range(NCORES)),
                                   trace=False)
    LAST_EXEC_NS = res.exec_time_ns
    LAST_RES = res
    outT = [res.results[c]["outT"] for c in range(NCORES)]
    halves = []
    for b in range(B):
        acc = outT[4 * b].astype(np.float64)
        for c in range(4 * b + 1, 4 * b + 4):
            acc = acc + outT[c]
        halves.append(acc.T)
    return np.stack(halves).astype(np.float32)
